# revision 22
# baseline (speedup 1.0000x reference)
"""Trainium2 Bass kernel for causal multi-head attention.

Problem: B=2, C=2048, H=1024, 16 heads, head_dim=64, float32.
    qkv = x @ Wqkv.T + b ; causal softmax attention ; out = att @ Wo.T + b

Sharding over 8 NeuronCores: core i owns heads {2i, 2i+1} for BOTH batches
(tensor parallel over heads). Attention outputs are redistributed with four
staggered AllToAlls so almost all collective time hides under compute:
batch 0 (512KB, after batch-0 attention, hidden under batch-1 attention),
batch-1 qt{0,1} (256KB, hidden under tiles 2-3), qt{2} (128KB, hidden
under tile 3), qt{3} (128KB, exposed, overlapped with the deferred half of
batch-0's output projection). Core c owns batch-0 tokens
[512*(c%4)+256*(c//4), +256) and batch-1 tokens 512*qt+64c..+64 per qt.

Performance structure:
  - All big loads are single-DMA (host pre-tiles xT/weights to match the
    SBUF layout) and all 8 x tiles are prefetched up front.
  - The attention inner loop is exp-gated on ScalarE (~1.1us per 128-k
    block); QKV matmuls for later tiles and the batch-0 output projection
    are interleaved one-per-k-block into those gaps (generator fillers).
  - Diagonal tiles: exp is range-limited to unmasked columns, and the
    causal mask multiply covers only the 128-wide triangle band (identical
    band for every diagonal offset).
"""
import math
import numpy as np

C, H, NH, HD = 2048, 1024, 16, 64
B = 2
NCORES = 8

_cache = {}


def _build():
    import concourse.bass as bass
    import concourse.bacc as bacc
    import concourse.tile as tile
    import concourse.mybir as mybir

    dt = mybir.dt
    f32 = dt.float32
    f32r = dt.float32r
    bf16 = dt.bfloat16
    AF = mybir.ActivationFunctionType

    nc = bacc.Bacc("TRN2", target_bir_lowering=False, debug=False,
                   enable_asserts=True, num_devices=NCORES)

    def din(name, shape, d=f32):
        return nc.dram_tensor(name, shape, d, kind="ExternalInput").ap()

    xtiles = din("xtiles", [1024, 4096], bf16)     # 8 tiles [128, 8*512]
    wqkT = din("wqkT", [128, 2048], bf16)          # hb-major [128, 8*256]
    qk_bias = din("qk_bias", [128, 2])
    wvT = din("wvT", [128, 1024], bf16)            # hb-major [128, 8*128]
    vb_bcast = din("vb_bcast", [128, 128])
    tri2 = din("tri2", [128, 256], bf16)           # causal triangle, x2
    ident = din("ident", [128, 128], bf16)
    ones64 = din("ones64", [128, 64], f32r)
    onesbf = din("onesbf", [128, 64], bf16)
    woT = din("woT", [128, 8192], bf16)            # cb-major [128, 8*1024]
    wob_bcast = din("wob_bcast", [128, 1024])
    y_out = nc.dram_tensor("y", [512, 1024], f32, kind="ExternalOutput").ap()

    with tile.TileContext(nc) as tc:
        ctx_lp = nc.allow_low_precision(
            reason="float32r/bf16 operands; all matmuls accumulate in f32 PSUM")
        ctx_lp.__enter__()
        with (
            tc.tile_pool(name="const", bufs=1) as const_pool,
            tc.tile_pool(name="persist", bufs=1) as persist,
            tc.tile_pool(name="dram", bufs=1, space="DRAM") as dram,
            tc.tile_pool(name="qkvps", bufs=2, space="PSUM") as qkv_ps,
            tc.tile_pool(name="sps", bufs=2, space="PSUM") as s_ps,
            tc.tile_pool(name="avps", bufs=1, space="PSUM") as av_ps,
            tc.tile_pool(name="xt", bufs=1) as xt_pool,
            tc.tile_pool(name="psb", bufs=8) as p_pool,
            tc.tile_pool(name="attsb", bufs=4) as att_pool,
            tc.tile_pool(name="recsb", bufs=4) as rec_pool,
            tc.tile_pool(name="ysb", bufs=4) as y_pool,
        ):
            # -------- weights + all x tiles (sync queue, critical path first)
            wqk_sb = const_pool.tile([128, 8 * 256], bf16, tag="wqk")
            nc.sync.dma_start(wqk_sb[:], wqkT)
            xts = []
            for tt in range(8):
                xts.append(xt_pool.tile([128, 8 * 512], bf16, tag=f"xt{tt}",
                                        name=f"xt{tt}"))
            nc.sync.dma_start(xts[0][:], xtiles[0:128, :])
            wv_sb = const_pool.tile([128, 8 * 128], bf16, tag="wv")
            nc.sync.dma_start(wv_sb[:], wvT)
            for tt in range(1, 8):
                nc.sync.dma_start(xts[tt][:], xtiles[128 * tt:128 * tt + 128, :])

            # -------- persistent activations
            qT_sb = persist.tile([128, 4096], bf16, tag="qT")
            kT_sb = persist.tile([128, 4096], bf16, tag="kT")
            vT_sb = persist.tile([128, 4096], bf16, tag="vT")
            v_sb = persist.tile([128, 32 * 130], bf16, tag="v")   # [t, d|1]
            att_sb0 = persist.tile([128, 8 * 256], bf16, tag="attsb0")
            att_sb1 = persist.tile([128, 8 * 256], bf16, tag="attsb1")

            # -------- constants needed later (gpsimd queue, off critical path)
            qkb_sb = const_pool.tile([128, 2], f32, tag="qkb")
            nc.gpsimd.dma_start(qkb_sb[:], qk_bias)
            vbb_sb = const_pool.tile([128, 128], f32, tag="vbb")
            nc.gpsimd.dma_start(vbb_sb[:], vb_bcast)
            ident_sb = const_pool.tile([128, 128], bf16, tag="ident")
            nc.gpsimd.dma_start(ident_sb[:], ident)
            tri2_sb = const_pool.tile([128, 256], bf16, tag="tri2")
            nc.gpsimd.dma_start(tri2_sb[:], tri2)
            ones64_sb = const_pool.tile([128, 64], f32r, tag="ones64")
            nc.gpsimd.dma_start(ones64_sb[:], ones64)
            onesbf_sb = const_pool.tile([128, 64], bf16, tag="onesbf")
            nc.gpsimd.dma_start(onesbf_sb[:], onesbf)
            wob_sb = const_pool.tile([128, 1024], f32, tag="wob")
            nc.gpsimd.dma_start(wob_sb[:], wob_bcast)
            wo_sb = const_pool.tile([128, 8 * 1024], bf16, tag="wot")
            nc.gpsimd.dma_start(wo_sb[:], woT)

            # ones columns of v_sb: cols {130*s + 64, 130*s + 129}
            ones_view = v_sb[:].rearrange("p (s h e) -> p s h e", s=32, h=2, e=65)
            nc.vector.tensor_copy(
                ones_view[:, :, :, 64],
                onesbf_sb[:].rearrange("p (s h) -> p s h", s=32, h=2))

            a2a_in0 = dram.tile([1024, 256], bf16, tag="a2a_in0")
            a2a_out0 = dram.tile([1024, 256], bf16, tag="a2a_out0")
            # batch-1 A2A split in three staggered collectives:
            # qt {0,1} -> in1a, qt {2} -> in1b, qt {3} -> in1c
            a2a_in1a = dram.tile([1024, 128], bf16, tag="a2a_in1a")
            a2a_out1a = dram.tile([1024, 128], bf16, tag="a2a_out1a")
            a2a_in1b = dram.tile([1024, 64], bf16, tag="a2a_in1b")
            a2a_out1b = dram.tile([1024, 64], bf16, tag="a2a_out1b")
            a2a_in1c = dram.tile([1024, 64], bf16, tag="a2a_in1c")
            a2a_out1c = dram.tile([1024, 64], bf16, tag="a2a_out1c")

            # ---------------- building blocks ----------------
            def qkv_gen(tt):
                """QKV projection for one 512-wide t tile; yields per matmul."""
                xt = xts[tt]
                for ob in range(2):
                    ps = qkv_ps.tile([128, 512], f32, tag="qkv", name="qkvp")
                    for hb in range(8):
                        nc.tensor.matmul(
                            ps[:],
                            wqk_sb[:, 256 * hb + 128 * ob:256 * hb + 128 * ob + 128],
                            xt[:, 512 * hb:512 * hb + 512],
                            start=(hb == 0), stop=(hb == 7))
                        yield
                    dest = qT_sb if ob == 0 else kT_sb
                    nc.vector.tensor_scalar_add(
                        dest[:, 512 * tt:512 * tt + 512], ps[:],
                        qkb_sb[:, ob:ob + 1])
                ps = qkv_ps.tile([128, 512], f32, tag="qkv", name="qkvp2")
                for hb in range(8):
                    nc.tensor.matmul(
                        ps[:],
                        wv_sb[:, 128 * hb:128 * hb + 128],
                        xt[:, 512 * hb:512 * hb + 512],
                        start=(hb == 0), stop=(hb == 7))
                    yield
                nc.vector.tensor_copy(vT_sb[:, 512 * tt:512 * tt + 512], ps[:])

            def wo_gen(b, att_sb, qhs=(0, 1)):
                """Output projection for one batch's 256 tokens; yields/matmul."""
                for qh in qhs:
                    ysb = y_pool.tile([128, 1024], f32, tag="ysb", name="ysb")
                    for jt in range(2):
                        ps = qkv_ps.tile([128, 512], f32, tag="qkv", name="yps")
                        for cb in range(8):
                            nc.tensor.matmul(
                                ps[:],
                                att_sb[:, 256 * cb + 128 * qh:256 * cb + 128 * qh + 128],
                                wo_sb[:, 1024 * cb + 512 * jt:1024 * cb + 512 * jt + 512],
                                start=(cb == 0), stop=(cb == 7))
                            yield
                        nc.vector.tensor_add(
                            ysb[:, 512 * jt:512 * jt + 512], ps[:],
                            wob_sb[:, 512 * jt:512 * jt + 512])
                        nc.sync.dma_start(
                            y_out[256 * b + 128 * qh:256 * b + 128 * qh + 128,
                                  512 * jt:512 * jt + 512],
                            ysb[:, 512 * jt:512 * jt + 512])

            def drain(gen):
                for _ in gen:
                    pass

            class Fill:
                """FIFO of filler generators. The PE p-state ramp resets on
                any idle gap (2.4GHz only after 3us of continuous execution),
                so every dependency stall is plugged with deferred matmuls."""
                def __init__(self):
                    self.gens = []

                def add(self, g):
                    self.gens.append(g)

                def take(self, n):
                    while n > 0 and self.gens:
                        try:
                            next(self.gens[0])
                            n -= 1
                        except StopIteration:
                            self.gens.pop(0)

                def finish(self, g):
                    while g in self.gens:
                        try:
                            next(g)
                        except StopIteration:
                            self.gens.remove(g)

                def flush(self):
                    while self.gens:
                        self.finish(self.gens[0])

            def v_transpose(b, tbs):
                """v^T -> v tiles [128 t, 128 d] + bias, into bf16 v_sb."""
                for tb in tbs:
                    slot = 16 * b + tb
                    ps = qkv_ps.tile([128, 128], bf16, tag="qkv", name="vt")
                    nc.tensor.transpose(
                        ps[:], vT_sb[:, 2048 * b + 128 * tb:2048 * b + 128 * tb + 128],
                        ident_sb[:])
                    dv = v_sb[:].rearrange("p (s h e) -> p s h e", s=32, h=2, e=65)
                    sv = ps[:].rearrange("p (h e) -> p h e", h=2, e=64)
                    bv = vbb_sb[:].rearrange("p (h e) -> p h e", h=2, e=64)
                    nc.vector.tensor_add(dv[:, slot, :, 0:64], sv, bv)

            tri2_v = tri2_sb[:].rearrange("p (h q) -> p h q", h=2)

            def attention_qtile(b, qt, fill=None, per_slot=2):
                nkb = 4 * (qt + 1)
                avA = av_ps.tile([65, 512], f32, tag="avA", name="avA")
                avB = av_ps.tile([65, 512], f32, tag="avB", name="avB")
                qlo = 2048 * b + 512 * qt
                pend = None   # software pipeline: AV runs one kb behind S^T/exp
                for kb in range(nkb):
                    klo = 2048 * b + 128 * kb
                    roff = kb - 4 * qt
                    # columns q < 128*roff of a diagonal tile are fully masked:
                    # skip them in the S^T, exp and AV streams
                    lo = 128 * roff if roff > 0 else 0
                    sAB = s_ps.tile([128, 1024], f32, tag="s", name="sAB")
                    nc.tensor.matmul(
                        sAB[:, lo:512], kT_sb[0:64, klo:klo + 128],
                        qT_sb[0:64, qlo + lo:qlo + 512])
                    nc.tensor.matmul(
                        sAB[:, 512 + lo:1024], kT_sb[64:128, klo:klo + 128],
                        qT_sb[64:128, qlo + lo:qlo + 512])
                    pAB = p_pool.tile([128, 1024], bf16, tag="p", name="pAB")
                    if lo:
                        nc.scalar.activation(
                            pAB[:].rearrange("p (h q) -> p h q", h=2)[:, :, lo:512],
                            sAB[:].rearrange("p (h q) -> p h q", h=2)[:, :, lo:512],
                            AF.Exp, scale=1.0 / math.sqrt(HD))
                    else:
                        nc.scalar.activation(pAB[:], sAB[:], AF.Exp,
                                             scale=1.0 / math.sqrt(HD))
                    if fill is not None:
                        fill.take(per_slot)
                    if roff >= 0:
                        # only the 128-wide diagonal band needs masking
                        pv = pAB[:].rearrange("p (h q) -> p h q", h=2)
                        nc.vector.tensor_mul(pv[:, :, lo:lo + 128],
                                             pv[:, :, lo:lo + 128], tri2_v)
                    if pend is not None:
                        pkb, ppAB, plo = pend
                        pslot = 16 * b + pkb
                        nc.tensor.matmul(
                            avA[:, plo:512], v_sb[:, 130 * pslot:130 * pslot + 65],
                            ppAB[:, plo:512], start=(pkb == 0), stop=False)
                        nc.tensor.matmul(
                            avB[:, plo:512], v_sb[:, 130 * pslot + 65:130 * pslot + 130],
                            ppAB[:, 512 + plo:1024], start=(pkb == 0), stop=False)
                    pend = (kb, pAB, lo)
                pkb, ppAB, plo = pend
                pslot = 16 * b + pkb
                nc.tensor.matmul(
                    avA[:, plo:512], v_sb[:, 130 * pslot:130 * pslot + 65],
                    ppAB[:, plo:512], start=(pkb == 0), stop=True)
                nc.tensor.matmul(
                    avB[:, plo:512], v_sb[:, 130 * pslot + 65:130 * pslot + 130],
                    ppAB[:, 512 + plo:1024], start=(pkb == 0), stop=True)
                if fill is not None:
                    fill.take(2)
                # normalize + ship to this batch's a2a buffer(s).
                for h, av in ((0, avA), (1, avB)):
                    lrow = rec_pool.tile([128, 512], f32r, tag="lrow", name="lrow")
                    nc.vector.tensor_copy(lrow[64:65, :], av[64:65, :])
                    rcp = s_ps.tile([64, 512], f32, tag="s", name="rcp")
                    nc.tensor.matmul(rcp[:], ones64_sb[64:65, :], lrow[64:65, :])
                    if fill is not None:
                        fill.take(2)
                    rcp_sb = rec_pool.tile([64, 512], f32, tag="recsb",
                                           name="rcp_sb")
                    nc.vector.reciprocal_approx_fast(rcp_sb[:], rcp[:])
                    att = att_pool.tile([64, 512], bf16, tag="att", name="att")
                    nc.vector.tensor_mul(att[:], av[0:64, :], rcp_sb[:])
                    if b == 0:
                        # q 0:256 -> chunk qt, q 256:512 -> chunk qt+4
                        a2a_v = a2a_in0[:].rearrange(
                            "(c2 c1 r) q -> c1 r c2 q", c2=2, c1=4, r=128)
                        nc.sync.dma_start(
                            a2a_v[qt, 64 * h:64 * h + 64, :, :],
                            att[:].rearrange("p (c q) -> p c q", c=2))
                    elif qt < 2:
                        # chunk c gets q 64c:64c+64; col block = qt
                        a2a_v = a2a_in1a[:].rearrange(
                            "(c r) (t q) -> t r c q", c=8, r=128, t=2)
                        nc.sync.dma_start(
                            a2a_v[qt, 64 * h:64 * h + 64, :, :],
                            att[:].rearrange("p (c q) -> p c q", c=8))
                    else:
                        buf = a2a_in1b if qt == 2 else a2a_in1c
                        a2a_v = buf[:].rearrange("(c r) q -> r c q", c=8, r=128)
                        nc.sync.dma_start(
                            a2a_v[64 * h:64 * h + 64, :, :],
                            att[:].rearrange("p (c q) -> p c q", c=8))

            def a2a(a2a_in, a2a_out):
                nc.gpsimd.collective_compute(
                    "AllToAll", mybir.AluOpType.bypass,
                    replica_groups=[list(range(NCORES))],
                    ins=[a2a_in[:].opt()], outs=[a2a_out[:].opt()])

            def att_gather(att_sb, a2a_out, col0, w):
                nc.gpsimd.dma_start(
                    att_sb[:].rearrange("p (c q) -> p c q", c=8)[:, :, col0:col0 + w],
                    a2a_out[:].rearrange("(c p) q -> p c q", c=8, p=128))

            # ---------------- emission: interleaved schedule ----------------
            fs = Fill()
            drain(qkv_gen(0))
            v_transpose(0, range(0, 4))
            g1, g2, g3, g4 = (qkv_gen(t) for t in (1, 2, 3, 4))
            fs.add(g1)
            attention_qtile(0, 0, fill=fs)
            fs.finish(g1)
            v_transpose(0, range(4, 8))
            fs.add(g2)
            attention_qtile(0, 1, fill=fs)
            fs.finish(g2)
            v_transpose(0, range(8, 12))
            fs.add(g3)
            fs.add(g4)
            attention_qtile(0, 2, fill=fs)
            fs.finish(g3)
            v_transpose(0, range(12, 16))
            attention_qtile(0, 3, fill=fs)
            fs.finish(g4)
            a2a(a2a_in0, a2a_out0)
            att_gather(att_sb0, a2a_out0, 0, 256)
            v_transpose(1, range(0, 4))
            g5, g6, g7 = (qkv_gen(t) for t in (5, 6, 7))
            fs.add(g5)
            attention_qtile(1, 0, fill=fs)
            fs.finish(g5)
            v_transpose(1, range(4, 8))
            fs.add(g6)
            attention_qtile(1, 1, fill=fs)
            fs.finish(g6)
            v_transpose(1, range(8, 12))
            a2a(a2a_in1a, a2a_out1a)
            att_gather(att_sb1, a2a_out1a, 0, 128)
            gw0 = wo_gen(0, att_sb0)
            fs.add(g7)
            fs.add(gw0)
            attention_qtile(1, 2, fill=fs)
            fs.finish(g7)
            v_transpose(1, range(12, 16))
            a2a(a2a_in1b, a2a_out1b)
            att_gather(att_sb1, a2a_out1b, 128, 64)
            fs.add(wo_gen(1, att_sb1, qhs=(0,)))
            attention_qtile(1, 3, fill=fs, per_slot=1)
            a2a(a2a_in1c, a2a_out1c)
            fs.flush()  # leftover Wo work runs while the last A2A flies
            att_gather(att_sb1, a2a_out1c, 192, 64)
            drain(wo_gen(1, att_sb1, qhs=(1,)))
        ctx_lp.__exit__(None, None, None)

    nc.compile()
    return nc


def host_prep(x, Wqkv_w, Wqkv_b, Wo_w, Wo_b):
    import ml_dtypes
    bf16 = ml_dtypes.bfloat16

    x = np.asarray(x, np.float32)
    Wqkv_w = np.asarray(Wqkv_w, np.float32)
    Wqkv_b = np.asarray(Wqkv_b, np.float32)
    Wo_w = np.asarray(Wo_w, np.float32)
    Wo_b = np.asarray(Wo_b, np.float32)

    # xtiles[128*tt + p, 512*hb + c] = xT[128*hb + p, 512*tt + c]
    # where xT = concat(x[0].T, x[1].T) [1024, 4096]
    xT = np.concatenate([x[0].T, x[1].T], axis=1)          # [1024, 4096]
    xtiles = np.ascontiguousarray(
        xT.reshape(8, 128, 8, 512).transpose(2, 1, 0, 3).reshape(1024, 4096)
    ).astype(bf16)

    tri = (np.arange(128)[:, None] <= np.arange(128)[None, :])
    tri2 = np.ascontiguousarray(
        np.concatenate([tri, tri], axis=1).astype(np.float32)).astype(bf16)
    ident = np.eye(128, dtype=np.float32).astype(bf16)
    ones64 = np.ones((128, 64), np.float32)
    onesbf = np.ones((128, 64), bf16)
    # woT2[p, 1024*cb + j] = Wo_w[j, 128*cb + p]
    woT2 = np.ascontiguousarray(
        Wo_w.T.reshape(8, 128, 1024).transpose(1, 0, 2).reshape(128, 8192)
    ).astype(bf16)
    wob_bcast = np.tile(Wo_b[None, :], (128, 1)).astype(np.float32)

    in_maps = []
    for i in range(NCORES):
        hA, hB = 2 * i, 2 * i + 1
        rows_qk = np.r_[64 * hA:64 * hA + 64, 64 * hB:64 * hB + 64,
                        1024 + 64 * hA:1024 + 64 * hA + 64,
                        1024 + 64 * hB:1024 + 64 * hB + 64]
        # wqkT2[p, 256*hb + j] = Wqkv_w[rows_qk].T[128*hb + p, j]
        wqkT = Wqkv_w[rows_qk].T                            # [1024, 256]
        wqkT2 = np.ascontiguousarray(
            wqkT.reshape(8, 128, 256).transpose(1, 0, 2).reshape(128, 2048)
        ).astype(bf16)
        qkb = np.ascontiguousarray(Wqkv_b[rows_qk].reshape(2, 128).T)
        rows_v = np.r_[2048 + 64 * hA:2048 + 64 * hA + 64,
                       2048 + 64 * hB:2048 + 64 * hB + 64]
        wvT = Wqkv_w[rows_v].T                              # [1024, 128]
        wvT2 = np.ascontiguousarray(
            wvT.reshape(8, 128, 128).transpose(1, 0, 2).reshape(128, 1024)
        ).astype(bf16)
        vbb = np.tile(Wqkv_b[rows_v][None, :], (128, 1)).astype(np.float32)
        in_maps.append(dict(
            xtiles=xtiles, wqkT=wqkT2, qk_bias=qkb, wvT=wvT2, vb_bcast=vbb,
            tri2=tri2, ident=ident, ones64=ones64, onesbf=onesbf, woT=woT2,
            wob_bcast=wob_bcast))
    return in_maps


def _ensure_ntff_hook_module():
    """run_bass_kernel_spmd(trace=True) under axon imports
    antenv.axon_hooks; provide a working ctypes-based fallback if the
    environment doesn't ship one so tracing (e.g. via BASS_TRACE=1) works."""
    import importlib
    import sys
    import types
    try:
        importlib.import_module("antenv.axon_hooks")
        return
    except ImportError:
        pass
    import contextlib
    import ctypes

    mod = types.ModuleType("antenv.axon_hooks")
    state = {"hook": None}

    def set_axon_ntff_profile_hook(h):
        state["hook"] = h

    def _make():
        try:
            lib = ctypes.CDLL("/opt/axon/libaxon_pjrt.so")
        except OSError:
            return None
        if not hasattr(lib, "axon_start_nrt_profile"):
            return None
        lib.axon_start_nrt_profile.argtypes = [
            ctypes.POINTER(ctypes.c_int64), ctypes.c_size_t]
        lib.axon_start_nrt_profile.restype = ctypes.c_int64
        lib.axon_stop_nrt_profile.argtypes = [ctypes.c_char_p]
        lib.axon_stop_nrt_profile.restype = ctypes.c_int64

        @contextlib.contextmanager
        def _hook(output_dir, device_ids):
            import jax
            jax.devices()
            if device_ids:
                ids = (ctypes.c_int64 * len(device_ids))(*device_ids)
                rc = lib.axon_start_nrt_profile(ids, len(device_ids))
            else:
                rc = lib.axon_start_nrt_profile(None, 0)
            if rc != 0:
                raise RuntimeError(f"axon_start_nrt_profile rc={rc}")
            try:
                yield
            finally:
                lib.axon_stop_nrt_profile(str(output_dir).encode())

        return _hook

    def get_axon_ntff_profile_hook():
        if state["hook"] is None:
            state["hook"] = _make()
        return state["hook"]

    mod.set_axon_ntff_profile_hook = set_axon_ntff_profile_hook
    mod.get_axon_ntff_profile_hook = get_axon_ntff_profile_hook
    try:
        import antenv
        sys.modules["antenv.axon_hooks"] = mod
        antenv.axon_hooks = mod
    except ImportError:
        pass


def kernel(x, Wqkv_w, Wqkv_b, Wo_w, Wo_b):
    from concourse import bass_utils

    _ensure_ntff_hook_module()

    if "nc" not in _cache:
        _cache["nc"] = _build()
    nc = _cache["nc"]

    in_maps = host_prep(x, Wqkv_w, Wqkv_b, Wo_w, Wo_b)
    res = bass_utils.run_bass_kernel_spmd(nc, in_maps, core_ids=list(range(NCORES)))
    _cache["last_results"] = res

    out = np.zeros((B, C, H), np.float32)
    for c in range(NCORES):
        y = res.results[c]["y"]
        lo = 512 * (c % 4) + 256 * (c // 4)
        out[0, lo:lo + 256, :] = y[0:256]
        for qt in range(4):
            out[1, 512 * qt + 64 * c:512 * qt + 64 * c + 64, :] = \
                y[256 + 64 * qt:256 + 64 * qt + 64]
    return out


# revision 23
# speedup vs baseline: 1.0310x; 1.0310x over previous
"""Trainium2 Bass kernel for causal multi-head attention.

Problem: B=2, C=2048, H=1024, 16 heads, head_dim=64, float32.
    qkv = x @ Wqkv.T + b ; causal softmax attention ; out = att @ Wo.T + b

Sharding over 8 NeuronCores: core i owns heads {2i, 2i+1} for BOTH batches
(tensor parallel over heads). Attention outputs are redistributed with four
staggered AllToAlls so almost all collective time hides under compute:
batch 0 (512KB, after batch-0 attention, hidden under batch-1 attention),
batch-1 qt{0,1} (256KB, hidden under tiles 2-3), qt{2} (128KB, hidden
under tile 3), qt{3} (128KB, exposed, overlapped with the deferred half of
batch-0's output projection). Core c owns batch-0 tokens
[512*(c%4)+256*(c//4), +256) and batch-1 tokens 512*qt+64c..+64 per qt.

Performance structure:
  - All big loads are single-DMA (host pre-tiles xT/weights to match the
    SBUF layout) and all 8 x tiles are prefetched up front.
  - The attention inner loop is exp-gated on ScalarE (~1.1us per 128-k
    block); QKV matmuls for later tiles and the batch-0 output projection
    are interleaved one-per-k-block into those gaps (generator fillers).
  - Diagonal tiles: exp is range-limited to unmasked columns, and the
    causal mask multiply covers only the 128-wide triangle band (identical
    band for every diagonal offset).
"""
import math
import numpy as np

C, H, NH, HD = 2048, 1024, 16, 64
B = 2
NCORES = 8

_cache = {}


def _build():
    import concourse.bass as bass
    import concourse.bacc as bacc
    import concourse.tile as tile
    import concourse.mybir as mybir

    dt = mybir.dt
    f32 = dt.float32
    f32r = dt.float32r
    bf16 = dt.bfloat16
    AF = mybir.ActivationFunctionType

    nc = bacc.Bacc("TRN2", target_bir_lowering=False, debug=False,
                   enable_asserts=True, num_devices=NCORES)

    def din(name, shape, d=f32):
        return nc.dram_tensor(name, shape, d, kind="ExternalInput").ap()

    xtiles = din("xtiles", [1024, 4096], bf16)     # 8 tiles [128, 8*512]
    wqkT = din("wqkT", [128, 2048], bf16)          # hb-major [128, 8*256]
    qk_bias = din("qk_bias", [128, 2])
    wvT = din("wvT", [128, 1024], bf16)            # hb-major [128, 8*128]
    vb_bcast = din("vb_bcast", [128, 128])
    tri2 = din("tri2", [128, 256], bf16)           # causal triangle, x2
    ident = din("ident", [128, 128], bf16)
    ones64 = din("ones64", [128, 64], f32r)
    onesbf = din("onesbf", [128, 64], bf16)
    woT = din("woT", [128, 8192], bf16)            # cb-major [128, 8*1024]
    wob_bcast = din("wob_bcast", [128, 1024])
    y_out = nc.dram_tensor("y", [512, 1024], f32, kind="ExternalOutput").ap()

    with tile.TileContext(nc) as tc:
        ctx_lp = nc.allow_low_precision(
            reason="float32r/bf16 operands; all matmuls accumulate in f32 PSUM")
        ctx_lp.__enter__()
        with (
            tc.tile_pool(name="const", bufs=1) as const_pool,
            tc.tile_pool(name="persist", bufs=1) as persist,
            tc.tile_pool(name="dram", bufs=1, space="DRAM") as dram,
            tc.tile_pool(name="qkvps", bufs=2, space="PSUM") as qkv_ps,
            tc.tile_pool(name="sps", bufs=2, space="PSUM") as s_ps,
            tc.tile_pool(name="avps", bufs=1, space="PSUM") as av_ps,
            tc.tile_pool(name="xt", bufs=1) as xt_pool,
            tc.tile_pool(name="psb", bufs=8) as p_pool,
            tc.tile_pool(name="attsb", bufs=4) as att_pool,
            tc.tile_pool(name="recsb", bufs=4) as rec_pool,
            tc.tile_pool(name="ysb", bufs=4) as y_pool,
        ):
            # -------- weights + all x tiles (sync queue, critical path first)
            wqk_sb = const_pool.tile([128, 8 * 256], bf16, tag="wqk")
            nc.sync.dma_start(wqk_sb[:], wqkT)
            xts = []
            for tt in range(8):
                xts.append(xt_pool.tile([128, 8 * 512], bf16, tag=f"xt{tt}",
                                        name=f"xt{tt}"))
            nc.sync.dma_start(xts[0][:], xtiles[0:128, :])
            wv_sb = const_pool.tile([128, 8 * 128], bf16, tag="wv")
            nc.sync.dma_start(wv_sb[:], wvT)
            for tt in range(1, 8):
                nc.sync.dma_start(xts[tt][:], xtiles[128 * tt:128 * tt + 128, :])

            # -------- persistent activations
            qT_sb = persist.tile([128, 4096], bf16, tag="qT")
            kT_sb = persist.tile([128, 4096], bf16, tag="kT")
            vT_sb = persist.tile([128, 4096], bf16, tag="vT")
            v_sb = persist.tile([128, 32 * 130], bf16, tag="v")   # [t, d|1]
            att_sb0 = persist.tile([128, 8 * 256], bf16, tag="attsb0")
            att_sb1 = persist.tile([128, 8 * 256], bf16, tag="attsb1")

            # -------- constants needed later (gpsimd queue, off critical path)
            qkb_sb = const_pool.tile([128, 2], f32, tag="qkb")
            nc.gpsimd.dma_start(qkb_sb[:], qk_bias)
            vbb_sb = const_pool.tile([128, 128], f32, tag="vbb")
            nc.gpsimd.dma_start(vbb_sb[:], vb_bcast)
            ident_sb = const_pool.tile([128, 128], bf16, tag="ident")
            nc.gpsimd.dma_start(ident_sb[:], ident)
            tri2_sb = const_pool.tile([128, 256], bf16, tag="tri2")
            nc.gpsimd.dma_start(tri2_sb[:], tri2)
            ones64_sb = const_pool.tile([128, 64], f32r, tag="ones64")
            nc.gpsimd.dma_start(ones64_sb[:], ones64)
            onesbf_sb = const_pool.tile([128, 64], bf16, tag="onesbf")
            nc.gpsimd.dma_start(onesbf_sb[:], onesbf)
            wob_sb = const_pool.tile([128, 1024], f32, tag="wob")
            nc.gpsimd.dma_start(wob_sb[:], wob_bcast)
            wo_sb = const_pool.tile([128, 8 * 1024], bf16, tag="wot")
            nc.gpsimd.dma_start(wo_sb[:], woT)

            # ones columns of v_sb: cols {130*s + 64, 130*s + 129}
            ones_view = v_sb[:].rearrange("p (s h e) -> p s h e", s=32, h=2, e=65)
            nc.vector.tensor_copy(
                ones_view[:, :, :, 64],
                onesbf_sb[:].rearrange("p (s h) -> p s h", s=32, h=2))

            a2a_in0 = dram.tile([1024, 256], bf16, tag="a2a_in0")
            a2a_out0 = dram.tile([1024, 256], bf16, tag="a2a_out0")
            # batch-1 A2A split in three staggered collectives:
            # qt {0,1} -> in1a, qt {2} -> in1b, qt {3} -> in1c
            a2a_in1a = dram.tile([1024, 128], bf16, tag="a2a_in1a")
            a2a_out1a = dram.tile([1024, 128], bf16, tag="a2a_out1a")
            a2a_in1b = dram.tile([1024, 64], bf16, tag="a2a_in1b")
            a2a_out1b = dram.tile([1024, 64], bf16, tag="a2a_out1b")
            a2a_in1c = dram.tile([1024, 64], bf16, tag="a2a_in1c")
            a2a_out1c = dram.tile([1024, 64], bf16, tag="a2a_out1c")

            # ---------------- building blocks ----------------
            def qkv_gen(tt):
                """QKV projection for one 512-wide t tile; yields per matmul."""
                xt = xts[tt]
                for ob in range(2):
                    ps = qkv_ps.tile([128, 512], f32, tag="qkv", name="qkvp")
                    for hb in range(8):
                        nc.tensor.matmul(
                            ps[:],
                            wqk_sb[:, 256 * hb + 128 * ob:256 * hb + 128 * ob + 128],
                            xt[:, 512 * hb:512 * hb + 512],
                            start=(hb == 0), stop=(hb == 7))
                        yield
                    dest = qT_sb if ob == 0 else kT_sb
                    nc.vector.tensor_scalar_add(
                        dest[:, 512 * tt:512 * tt + 512], ps[:],
                        qkb_sb[:, ob:ob + 1])
                ps = qkv_ps.tile([128, 512], f32, tag="qkv", name="qkvp2")
                for hb in range(8):
                    nc.tensor.matmul(
                        ps[:],
                        wv_sb[:, 128 * hb:128 * hb + 128],
                        xt[:, 512 * hb:512 * hb + 512],
                        start=(hb == 0), stop=(hb == 7))
                    yield
                nc.vector.tensor_copy(vT_sb[:, 512 * tt:512 * tt + 512], ps[:])

            def wo_gen(b, att_sb, qhs=(0, 1)):
                """Output projection for one batch's 256 tokens; yields/matmul."""
                for qh in qhs:
                    ysb = y_pool.tile([128, 1024], f32, tag="ysb", name="ysb")
                    for jt in range(2):
                        ps = qkv_ps.tile([128, 512], f32, tag="qkv", name="yps")
                        for cb in range(8):
                            nc.tensor.matmul(
                                ps[:],
                                att_sb[:, 256 * cb + 128 * qh:256 * cb + 128 * qh + 128],
                                wo_sb[:, 1024 * cb + 512 * jt:1024 * cb + 512 * jt + 512],
                                start=(cb == 0), stop=(cb == 7))
                            yield
                        nc.vector.tensor_add(
                            ysb[:, 512 * jt:512 * jt + 512], ps[:],
                            wob_sb[:, 512 * jt:512 * jt + 512])
                        nc.sync.dma_start(
                            y_out[256 * b + 128 * qh:256 * b + 128 * qh + 128,
                                  512 * jt:512 * jt + 512],
                            ysb[:, 512 * jt:512 * jt + 512])

            def drain(gen):
                for _ in gen:
                    pass

            class Fill:
                """FIFO of filler generators. The PE p-state ramp resets on
                any idle gap (2.4GHz only after 3us of continuous execution),
                so every dependency stall is plugged with deferred matmuls."""
                def __init__(self):
                    self.gens = []

                def add(self, g):
                    self.gens.append(g)

                def take(self, n):
                    while n > 0 and self.gens:
                        try:
                            next(self.gens[0])
                            n -= 1
                        except StopIteration:
                            self.gens.pop(0)

                def finish(self, g):
                    while g in self.gens:
                        try:
                            next(g)
                        except StopIteration:
                            self.gens.remove(g)

                def flush(self):
                    while self.gens:
                        self.finish(self.gens[0])

            def v_transpose(b, tbs):
                """v^T -> v tiles [128 t, 128 d] + bias, into bf16 v_sb."""
                for tb in tbs:
                    slot = 16 * b + tb
                    ps = qkv_ps.tile([128, 128], bf16, tag="qkv", name="vt")
                    nc.tensor.transpose(
                        ps[:], vT_sb[:, 2048 * b + 128 * tb:2048 * b + 128 * tb + 128],
                        ident_sb[:])
                    dv = v_sb[:].rearrange("p (s h e) -> p s h e", s=32, h=2, e=65)
                    sv = ps[:].rearrange("p (h e) -> p h e", h=2, e=64)
                    bv = vbb_sb[:].rearrange("p (h e) -> p h e", h=2, e=64)
                    nc.vector.tensor_add(dv[:, slot, :, 0:64], sv, bv)

            tri2_v = tri2_sb[:].rearrange("p (h q) -> p h q", h=2)

            def attention_qtile(b, qt, fill=None, per_slot=1):
                nkb = 4 * (qt + 1)
                avA = av_ps.tile([65, 512], f32, tag="avA", name="avA")
                avB = av_ps.tile([65, 512], f32, tag="avB", name="avB")
                qlo = 2048 * b + 512 * qt
                pend = None   # software pipeline: AV runs one kb behind S^T/exp
                for kb in range(nkb):
                    klo = 2048 * b + 128 * kb
                    roff = kb - 4 * qt
                    # columns q < 128*roff of a diagonal tile are fully masked:
                    # skip them in the S^T, exp and AV streams
                    lo = 128 * roff if roff > 0 else 0
                    sAB = s_ps.tile([128, 1024], f32, tag="s", name="sAB")
                    nc.tensor.matmul(
                        sAB[:, lo:512], kT_sb[0:64, klo:klo + 128],
                        qT_sb[0:64, qlo + lo:qlo + 512])
                    nc.tensor.matmul(
                        sAB[:, 512 + lo:1024], kT_sb[64:128, klo:klo + 128],
                        qT_sb[64:128, qlo + lo:qlo + 512])
                    pAB = p_pool.tile([128, 1024], bf16, tag="p", name="pAB")
                    if lo:
                        nc.scalar.activation(
                            pAB[:].rearrange("p (h q) -> p h q", h=2)[:, :, lo:512],
                            sAB[:].rearrange("p (h q) -> p h q", h=2)[:, :, lo:512],
                            AF.Exp, scale=1.0 / math.sqrt(HD))
                    else:
                        nc.scalar.activation(pAB[:], sAB[:], AF.Exp,
                                             scale=1.0 / math.sqrt(HD))
                    if fill is not None:
                        fill.take(per_slot)
                    if roff >= 0:
                        # only the 128-wide diagonal band needs masking
                        pv = pAB[:].rearrange("p (h q) -> p h q", h=2)
                        nc.vector.tensor_mul(pv[:, :, lo:lo + 128],
                                             pv[:, :, lo:lo + 128], tri2_v)
                    if pend is not None:
                        pkb, ppAB, plo = pend
                        pslot = 16 * b + pkb
                        nc.tensor.matmul(
                            avA[:, plo:512], v_sb[:, 130 * pslot:130 * pslot + 65],
                            ppAB[:, plo:512], start=(pkb == 0), stop=False)
                        nc.tensor.matmul(
                            avB[:, plo:512], v_sb[:, 130 * pslot + 65:130 * pslot + 130],
                            ppAB[:, 512 + plo:1024], start=(pkb == 0), stop=False)
                    pend = (kb, pAB, lo)
                pkb, ppAB, plo = pend
                pslot = 16 * b + pkb
                nc.tensor.matmul(
                    avA[:, plo:512], v_sb[:, 130 * pslot:130 * pslot + 65],
                    ppAB[:, plo:512], start=(pkb == 0), stop=True)
                nc.tensor.matmul(
                    avB[:, plo:512], v_sb[:, 130 * pslot + 65:130 * pslot + 130],
                    ppAB[:, 512 + plo:1024], start=(pkb == 0), stop=True)
                if fill is not None:
                    fill.take(2)
                # normalize + ship to this batch's a2a buffer(s).
                for h, av in ((0, avA), (1, avB)):
                    lrow = rec_pool.tile([128, 512], f32r, tag="lrow", name="lrow")
                    nc.vector.tensor_copy(lrow[64:65, :], av[64:65, :])
                    rcp = s_ps.tile([64, 512], f32, tag="s", name="rcp")
                    nc.tensor.matmul(rcp[:], ones64_sb[64:65, :], lrow[64:65, :])
                    if fill is not None:
                        fill.take(2)
                    rcp_sb = rec_pool.tile([64, 512], f32, tag="recsb",
                                           name="rcp_sb")
                    nc.vector.reciprocal_approx_fast(rcp_sb[:], rcp[:])
                    att = att_pool.tile([64, 512], bf16, tag="att", name="att")
                    nc.vector.tensor_mul(att[:], av[0:64, :], rcp_sb[:])
                    if b == 0:
                        # q 0:256 -> chunk qt, q 256:512 -> chunk qt+4
                        a2a_v = a2a_in0[:].rearrange(
                            "(c2 c1 r) q -> c1 r c2 q", c2=2, c1=4, r=128)
                        nc.sync.dma_start(
                            a2a_v[qt, 64 * h:64 * h + 64, :, :],
                            att[:].rearrange("p (c q) -> p c q", c=2))
                    elif qt < 2:
                        # chunk c gets q 64c:64c+64; col block = qt
                        a2a_v = a2a_in1a[:].rearrange(
                            "(c r) (t q) -> t r c q", c=8, r=128, t=2)
                        nc.sync.dma_start(
                            a2a_v[qt, 64 * h:64 * h + 64, :, :],
                            att[:].rearrange("p (c q) -> p c q", c=8))
                    else:
                        buf = a2a_in1b if qt == 2 else a2a_in1c
                        a2a_v = buf[:].rearrange("(c r) q -> r c q", c=8, r=128)
                        nc.sync.dma_start(
                            a2a_v[64 * h:64 * h + 64, :, :],
                            att[:].rearrange("p (c q) -> p c q", c=8))

            def a2a(a2a_in, a2a_out):
                nc.gpsimd.collective_compute(
                    "AllToAll", mybir.AluOpType.bypass,
                    replica_groups=[list(range(NCORES))],
                    ins=[a2a_in[:].opt()], outs=[a2a_out[:].opt()])

            def att_gather(att_sb, a2a_out, col0, w):
                nc.gpsimd.dma_start(
                    att_sb[:].rearrange("p (c q) -> p c q", c=8)[:, :, col0:col0 + w],
                    a2a_out[:].rearrange("(c p) q -> p c q", c=8, p=128))

            # ---------------- emission: interleaved schedule ----------------
            fs = Fill()
            drain(qkv_gen(0))
            v_transpose(0, range(0, 4))
            g1, g2, g3, g4 = (qkv_gen(t) for t in (1, 2, 3, 4))
            fs.add(g1)
            attention_qtile(0, 0, fill=fs)
            fs.finish(g1)
            v_transpose(0, range(4, 8))
            fs.add(g2)
            attention_qtile(0, 1, fill=fs)
            fs.finish(g2)
            v_transpose(0, range(8, 12))
            fs.add(g3)
            fs.add(g4)
            attention_qtile(0, 2, fill=fs)
            fs.finish(g3)
            v_transpose(0, range(12, 16))
            attention_qtile(0, 3, fill=fs)
            fs.finish(g4)
            a2a(a2a_in0, a2a_out0)
            att_gather(att_sb0, a2a_out0, 0, 256)
            v_transpose(1, range(0, 4))
            g5, g6, g7 = (qkv_gen(t) for t in (5, 6, 7))
            fs.add(g5)
            attention_qtile(1, 0, fill=fs)
            fs.finish(g5)
            v_transpose(1, range(4, 8))
            fs.add(g6)
            attention_qtile(1, 1, fill=fs)
            fs.finish(g6)
            v_transpose(1, range(8, 12))
            a2a(a2a_in1a, a2a_out1a)
            att_gather(att_sb1, a2a_out1a, 0, 128)
            gw0 = wo_gen(0, att_sb0)
            fs.add(g7)
            fs.add(gw0)
            attention_qtile(1, 2, fill=fs)
            fs.finish(g7)
            v_transpose(1, range(12, 16))
            a2a(a2a_in1b, a2a_out1b)
            att_gather(att_sb1, a2a_out1b, 128, 64)
            fs.add(wo_gen(1, att_sb1, qhs=(0,)))
            attention_qtile(1, 3, fill=fs, per_slot=1)
            a2a(a2a_in1c, a2a_out1c)
            fs.flush()  # leftover Wo work runs while the last A2A flies
            att_gather(att_sb1, a2a_out1c, 192, 64)
            drain(wo_gen(1, att_sb1, qhs=(1,)))
        ctx_lp.__exit__(None, None, None)

    nc.compile()
    return nc


def host_prep(x, Wqkv_w, Wqkv_b, Wo_w, Wo_b):
    import ml_dtypes
    bf16 = ml_dtypes.bfloat16

    x = np.asarray(x, np.float32)
    Wqkv_w = np.asarray(Wqkv_w, np.float32)
    Wqkv_b = np.asarray(Wqkv_b, np.float32)
    Wo_w = np.asarray(Wo_w, np.float32)
    Wo_b = np.asarray(Wo_b, np.float32)

    # xtiles[128*tt + p, 512*hb + c] = xT[128*hb + p, 512*tt + c]
    # where xT = concat(x[0].T, x[1].T) [1024, 4096]
    xT = np.concatenate([x[0].T, x[1].T], axis=1)          # [1024, 4096]
    xtiles = np.ascontiguousarray(
        xT.reshape(8, 128, 8, 512).transpose(2, 1, 0, 3).reshape(1024, 4096)
    ).astype(bf16)

    tri = (np.arange(128)[:, None] <= np.arange(128)[None, :])
    tri2 = np.ascontiguousarray(
        np.concatenate([tri, tri], axis=1).astype(np.float32)).astype(bf16)
    ident = np.eye(128, dtype=np.float32).astype(bf16)
    ones64 = np.ones((128, 64), np.float32)
    onesbf = np.ones((128, 64), bf16)
    # woT2[p, 1024*cb + j] = Wo_w[j, 128*cb + p]
    woT2 = np.ascontiguousarray(
        Wo_w.T.reshape(8, 128, 1024).transpose(1, 0, 2).reshape(128, 8192)
    ).astype(bf16)
    wob_bcast = np.tile(Wo_b[None, :], (128, 1)).astype(np.float32)

    in_maps = []
    for i in range(NCORES):
        hA, hB = 2 * i, 2 * i + 1
        rows_qk = np.r_[64 * hA:64 * hA + 64, 64 * hB:64 * hB + 64,
                        1024 + 64 * hA:1024 + 64 * hA + 64,
                        1024 + 64 * hB:1024 + 64 * hB + 64]
        # wqkT2[p, 256*hb + j] = Wqkv_w[rows_qk].T[128*hb + p, j]
        wqkT = Wqkv_w[rows_qk].T                            # [1024, 256]
        wqkT2 = np.ascontiguousarray(
            wqkT.reshape(8, 128, 256).transpose(1, 0, 2).reshape(128, 2048)
        ).astype(bf16)
        qkb = np.ascontiguousarray(Wqkv_b[rows_qk].reshape(2, 128).T)
        rows_v = np.r_[2048 + 64 * hA:2048 + 64 * hA + 64,
                       2048 + 64 * hB:2048 + 64 * hB + 64]
        wvT = Wqkv_w[rows_v].T                              # [1024, 128]
        wvT2 = np.ascontiguousarray(
            wvT.reshape(8, 128, 128).transpose(1, 0, 2).reshape(128, 1024)
        ).astype(bf16)
        vbb = np.tile(Wqkv_b[rows_v][None, :], (128, 1)).astype(np.float32)
        in_maps.append(dict(
            xtiles=xtiles, wqkT=wqkT2, qk_bias=qkb, wvT=wvT2, vb_bcast=vbb,
            tri2=tri2, ident=ident, ones64=ones64, onesbf=onesbf, woT=woT2,
            wob_bcast=wob_bcast))
    return in_maps


def _ensure_ntff_hook_module():
    """run_bass_kernel_spmd(trace=True) under axon imports
    antenv.axon_hooks; provide a working ctypes-based fallback if the
    environment doesn't ship one so tracing (e.g. via BASS_TRACE=1) works."""
    import importlib
    import sys
    import types
    try:
        importlib.import_module("antenv.axon_hooks")
        return
    except ImportError:
        pass
    import contextlib
    import ctypes

    mod = types.ModuleType("antenv.axon_hooks")
    state = {"hook": None}

    def set_axon_ntff_profile_hook(h):
        state["hook"] = h

    def _make():
        try:
            lib = ctypes.CDLL("/opt/axon/libaxon_pjrt.so")
        except OSError:
            return None
        if not hasattr(lib, "axon_start_nrt_profile"):
            return None
        lib.axon_start_nrt_profile.argtypes = [
            ctypes.POINTER(ctypes.c_int64), ctypes.c_size_t]
        lib.axon_start_nrt_profile.restype = ctypes.c_int64
        lib.axon_stop_nrt_profile.argtypes = [ctypes.c_char_p]
        lib.axon_stop_nrt_profile.restype = ctypes.c_int64

        @contextlib.contextmanager
        def _hook(output_dir, device_ids):
            import jax
            jax.devices()
            if device_ids:
                ids = (ctypes.c_int64 * len(device_ids))(*device_ids)
                rc = lib.axon_start_nrt_profile(ids, len(device_ids))
            else:
                rc = lib.axon_start_nrt_profile(None, 0)
            if rc != 0:
                raise RuntimeError(f"axon_start_nrt_profile rc={rc}")
            try:
                yield
            finally:
                lib.axon_stop_nrt_profile(str(output_dir).encode())

        return _hook

    def get_axon_ntff_profile_hook():
        if state["hook"] is None:
            state["hook"] = _make()
        return state["hook"]

    mod.set_axon_ntff_profile_hook = set_axon_ntff_profile_hook
    mod.get_axon_ntff_profile_hook = get_axon_ntff_profile_hook
    try:
        import antenv
        sys.modules["antenv.axon_hooks"] = mod
        antenv.axon_hooks = mod
    except ImportError:
        pass


def kernel(x, Wqkv_w, Wqkv_b, Wo_w, Wo_b):
    from concourse import bass_utils

    _ensure_ntff_hook_module()

    if "nc" not in _cache:
        _cache["nc"] = _build()
    nc = _cache["nc"]

    in_maps = host_prep(x, Wqkv_w, Wqkv_b, Wo_w, Wo_b)
    res = bass_utils.run_bass_kernel_spmd(nc, in_maps, core_ids=list(range(NCORES)))
    _cache["last_results"] = res

    out = np.zeros((B, C, H), np.float32)
    for c in range(NCORES):
        y = res.results[c]["y"]
        lo = 512 * (c % 4) + 256 * (c // 4)
        out[0, lo:lo + 256, :] = y[0:256]
        for qt in range(4):
            out[1, 512 * qt + 64 * c:512 * qt + 64 * c + 64, :] = \
                y[256 + 64 * qt:256 + 64 * qt + 64]
    return out
